# revision 40
# baseline (speedup 1.0000x reference)
"""Trainium2 Bass kernel for nn_BasicTransformer (B=4, T=1024, C=H=768,
vocab 50257, single-head causal attention + LM head).

Sharding: 8 cores = 4 batches x 2 vocab halves. Each core computes the
full embedding+attention for its batch (duplicated across the vocab pair,
~7% extra FLOPs) and the LM-head matmul for its vocab half. The LM head
(316 of 345 total GFLOP) dominates, so it is computed as
logitsT[v, t] = W_lm[:, v_tile].T @ att_outT with the weight tile
stationary and att_outT streaming from SBUF; W_lm streams from HBM once.

The LM head for tokens 128..1023 runs in fp8 (e4m3) with
perf_mode=DoubleRow (2 contraction rows per PE pass, ~1.44x bf16): W_lm
is converted bf16->fp8 on-chip scaled by 2^9, the attention output is
scaled by 2^10, and the PSUM result is descaled by 2^-19. Tokens 0..127
keep bf16 via a swapped-operand block (att-out stationary, W_lm moving)
because their attention outputs -- averages of few value rows -- are
large, and fp8 error there would breach the rel-err budget. Logits are
stored to HBM in bf16 and upcast host-side. Attention stays in bf16.
"""

import numpy as np

import concourse.bass as bass
import concourse.mybir as mybir
import concourse.tile as tile
from concourse import bacc, bass_utils
from concourse.masks import make_causal_mask, make_identity

P = 128
T = 1024          # sequence length
C = 768           # features == head size
KC = C // P       # 6 contraction chunks
TT = T // P       # 8 token tiles
VOCAB = 50257
VH = 25216        # padded vocab half (= 197 * 128), 2*VH >= VOCAB
VT = VH // P      # 197 vocab tiles per core
SCALE = float(C) ** -0.5
NEG = -1e30

F32 = mybir.dt.float32
F32R = mybir.dt.float32r
BF16 = mybir.dt.bfloat16
F8 = mybir.dt.float8e4
I32 = mybir.dt.int32

W_SCALE = 512.0       # weight pre-scale (keeps e4m3 out of subnormals)
O_SCALE = 1024.0      # on-chip att-out pre-scale
X_SCALE = 256.0       # embedding pre-scale for fp8 q/k projections
DESC = 1.0 / (W_SCALE * O_SCALE)

_CACHE = {}


def _build(with_bias):
    nc = bacc.Bacc("TRN2", target_bir_lowering=False, debug=False)

    idx_d = nc.dram_tensor("idx", [T], I32, kind="ExternalInput")
    wemb_d = nc.dram_tensor("W_embed", [VOCAB, C], F32, kind="ExternalInput")
    wpos_d = nc.dram_tensor("W_pos", [T, C], F32, kind="ExternalInput")
    wq_d = nc.dram_tensor("Wq", [C, C], BF16, kind="ExternalInput")
    wk_d = nc.dram_tensor("Wk", [C, C], BF16, kind="ExternalInput")
    wv_d = nc.dram_tensor("Wv", [C, C], BF16, kind="ExternalInput")
    wlm_d = nc.dram_tensor("W_lm", [C, VH], BF16, kind="ExternalInput")
    blm_d = nc.dram_tensor("b_lm", [VH], F32, kind="ExternalInput")
    out_d = nc.dram_tensor("logitsT", [VH, T], BF16, kind="ExternalOutput")
    out0_d = nc.dram_tensor("logits0", [P, VH], BF16, kind="ExternalOutput")

    with tile.TileContext(nc) as tc:
        _body(tc, nc, idx_d, wemb_d, wpos_d, wq_d, wk_d, wv_d, wlm_d,
              blm_d, out_d, out0_d, with_bias)
    nc.compile()
    return nc


def _body(tc, nc, idx_d, wemb_d, wpos_d, wq_d, wk_d, wv_d, wlm_d, blm_d,
          out_d, out0_d, with_bias):
    from contextlib import ExitStack

    with ExitStack() as ctx:
        const = ctx.enter_context(tc.tile_pool(name="const", bufs=1))

        ident = const.tile([P, P], F32)
        make_identity(nc, ident[:])
        cmask = const.tile([P, P], F32)
        make_causal_mask(nc, cmask[:], mask_val=NEG)
        ident_bf = const.tile([P, P], BF16)
        nc.vector.tensor_copy(ident_bf[:], ident[:])
        blm_s = const.tile([P, VT], F32)

        att_ctx = ExitStack()
        attp = att_ctx.enter_context(tc.tile_pool(name="attp", bufs=1))
        qT = attp.tile([P, KC, T], BF16)      # q transposed  [h, t]
        kT = attp.tile([P, KC, T], BF16)      # k transposed  [h, t]
        v_s = attp.tile([P, TT, C], BF16)     # v             [s, h]
        # att output transposed [h, t], fp8, one tile per h-PAIR so the
        # DoubleRow rhs AP [p, 2, t] comes from a single tile
        oT8 = [const.tile([P, 2, T], F8, name=f"oT8{j}")
               for j in range(KC // 2)]
        # bf16 att output for the first t-tile (t0 block of the LM head)
        oT0 = const.tile([P, KC, P], BF16, name="oT0")

        early_ctx = ExitStack()
        early = early_ctx.enter_context(tc.tile_pool(name="early", bufs=1))
        xT = early.tile([P, KC, T], BF16)     # x transposed  [c, t]
        xT8 = early.tile([P, KC, T], F8)      # scaled fp8 copy for q/k proj

        # ---- phase 1: embedding gather + positional add + transpose ----
        # All critical-path DMAs (idx -> gather, W_pos, Wq/Wk/Wv) are issued
        # ahead of compute so they enqueue before the W_lm prefetch stream.
        with tc.tile_pool(name="ph1", bufs=1) as ph1, \
             tc.tile_pool(name="ps1", bufs=4, space="PSUM") as ps1:
            # one DMA for all idx tiles and one for all of W_pos — fewer
            # serialized descriptors ahead of the gather stream
            idx_all = ph1.tile([P, TT], I32, tag="idx")
            nc.sync.dma_start(
                idx_all[:], idx_d.ap().rearrange("(t p) -> p t", p=P))
            xp_all = ph1.tile([P, TT, C], F32, tag="xp")
            nc.scalar.dma_start(
                xp_all[:], wpos_d.ap().rearrange("(t p) c -> p t c", p=P))
            # one gather per t-tile (multi-row offset gathers hang the
            # gpsimd drain; see round-3 trace)
            xgs = []
            for t in range(TT):
                xg = ph1.tile([P, C], F32, tag=f"xg{t}")
                nc.gpsimd.indirect_dma_start(
                    out=xg[:], out_offset=None, in_=wemb_d.ap()[:],
                    in_offset=bass.IndirectOffsetOnAxis(
                        ap=idx_all[:, t:t + 1], axis=0))
                xgs.append(xg)
            wq_s = early.tile([P, KC, C], BF16)
            wk_s = early.tile([P, KC, C], BF16)
            wv_s = early.tile([P, KC, C], BF16)
            for k in range(KC):
                for (w_s, w_d) in ((wq_s, wq_d), (wk_s, wk_d), (wv_s, wv_d)):
                    nc.sync.dma_start(
                        w_s[:, k],
                        w_d.ap()[k * P:(k + 1) * P, :])
            nc.sync.dma_start(blm_s[:], blm_d.ap().rearrange("(o p) -> p o", p=P))
            for t in range(TT):
                xb = ph1.tile([P, C], BF16, tag=f"xb{t}")
                nc.vector.tensor_add(out=xb[:], in0=xgs[t][:], in1=xp_all[:, t])
                for k in range(KC):
                    tp = ps1.tile([P, P], BF16, tag="tp")
                    nc.tensor.transpose(
                        tp[:], xb[:, k * P:(k + 1) * P], ident_bf[:])
                    nc.vector.tensor_copy(xT[:, k, t * P:(t + 1) * P], tp[:])
                    nc.vector.tensor_scalar_mul(
                        xT8[:, k, t * P:(t + 1) * P], tp[:], X_SCALE)

        # ---- phase 2: q/k/v projections (q/k in fp8 DoubleRow) ----
        wq8 = early.tile([P, KC, C], F8)
        wk8 = early.tile([P, KC, C], F8)
        nc.scalar.activation(wq8[:], wq_s[:],
                             mybir.ActivationFunctionType.Identity,
                             scale=W_SCALE)
        nc.scalar.activation(wk8[:], wk_s[:],
                             mybir.ActivationFunctionType.Identity,
                             scale=W_SCALE)
        QK_DESC = 1.0 / (X_SCALE * W_SCALE)
        with tc.tile_pool(name="ps2", bufs=6, space="PSUM") as ps2:
            for half in range(2):
                for (w8, dstT) in ((wq8, qT), (wk8, kT)):
                    for h in range(KC):
                        pt = ps2.tile([P, 512], F32, tag="qk")
                        for k in range(KC // 2):
                            nc.tensor.matmul(
                                pt[:],
                                w8[:, 2 * k:2 * k + 2, h * P:(h + 1) * P],
                                xT8[:, 2 * k:2 * k + 2,
                                    half * 512:(half + 1) * 512],
                                start=(k == 0), stop=(k == KC // 2 - 1),
                                perf_mode=mybir.MatmulPerfMode.DoubleRow)
                        nc.vector.tensor_scalar_mul(
                            dstT[:, h, half * 512:(half + 1) * 512], pt[:],
                            QK_DESC)
            for s in range(TT):
                for (n0, n1) in ((0, 512), (512, 768)):
                    pt = ps2.tile([P, 512], F32, tag="qk")
                    for k in range(KC):
                        nc.tensor.matmul(
                            pt[:, :n1 - n0],
                            xT[:, k, s * P:(s + 1) * P],
                            wv_s[:, k, n0:n1],
                            start=(k == 0), stop=(k == KC - 1))
                    nc.vector.tensor_copy(v_s[:, s, n0:n1], pt[:, :n1 - n0])
        early_ctx.close()

        # open the LM-head streaming pools now so W_lm prefetch DMAs can run
        # during the attention phases
        ph5_ctx = ExitStack()
        ph5 = ph5_ctx.enter_context(tc.tile_pool(name="ph5", bufs=5))
        out5 = ph5_ctx.enter_context(tc.tile_pool(name="out5", bufs=8))
        out5a = ph5_ctx.enter_context(tc.tile_pool(name="out5a", bufs=1))

        pT_ctx = ExitStack()
        pTp = pT_ctx.enter_context(tc.tile_pool(name="pTp", bufs=1))
        # attention probs transposed [s, t], one tile per s-chunk for finer deps
        pT = [pTp.tile([P, T], BF16, tag=f"pT{s}", name=f"pT{s}") for s in range(TT)]

        # zero the strictly-upper (future) blocks of pT that phase-4 matmuls
        # will read but phase 3 never writes
        for s in range(1, TT):
            lo = 0 if s < 4 else 512
            if s * P > lo:
                nc.vector.memset(pT[s][:, lo:s * P].bitcast(mybir.dt.uint16), 0)

        # ---- phases 3-5, interleaved ----
        # t-tile 0 (128 tokens) of the LM head is computed in bf16 with
        # swapped operands (att-out stationary, W_lm moving) because fp8
        # quantization error there is too large relative to the global logit
        # scale; t>=128 runs in fp8 e4m3 with perf_mode=DoubleRow (2
        # contraction rows per PE pass). Phase 4 is split around the second
        # half of phase 3, and chunk-0 LM work is woven between the phase-3
        # score groups so the PE has work during the softmax latency.
        CHUNK = 1024  # vocab columns per W_lm DMA chunk; VH = 24*1024 + 640
        offs = list(range(0, VH, CHUNK))
        PREF = 5      # W_lm chunks in flight (== ph5 "wlm" bufs)

        wl8p = pT_ctx.enter_context(tc.tile_pool(name="wl8p", bufs=3))
        out0p = pT_ctx.enter_context(tc.tile_pool(name="out0p", bufs=4))

        def issue_wlm(off):
            w = min(CHUNK, VH - off)
            wlb = ph5.tile([P, KC, CHUNK], BF16, tag="wlm", name=f"wlb{off}")
            nc.scalar.dma_start(
                wlb[:, :, :w],
                wlm_d.ap()[:, off:off + w].rearrange("(k p) n -> p k n", p=P))
            return wlb

        def convert_wlm(wlb, w):
            # pre-scaled fp8 copy for the DoubleRow sweep; per-pair ops on
            # scalar (vector is the busier engine) so the first MM group
            # isn't gated on one long op
            wl8 = wl8p.tile([P, KC, CHUNK], F8, tag="wl8")
            for pair in range(KC // 2):
                nc.scalar.activation(
                    wl8[:, 2 * pair:2 * pair + 2, :w],
                    wlb[:, 2 * pair:2 * pair + 2, :w],
                    mybir.ActivationFunctionType.Identity, scale=W_SCALE)
            return wl8

        # only chunk 0 up front: more W_lm prefetch here contends with the
        # embedding-gather DMAs and stalls the whole front of the kernel;
        # chunks 1..PREF-1 are issued one per t-tile during phase 3a
        wlbs = [issue_wlm(offs[0])]

        def scores(t, ph3, psX):
            L = (t + 1) * P
            srow = ph3.tile([P, T], F32, tag="srow", name=f"srow{t}")
            for b0 in range(0, L, 512):
                n = min(512, L - b0)
                pt = psX.tile([P, 512], F32, tag="sc")
                for k in range(KC):
                    nc.tensor.matmul(
                        pt[:, :n],
                        qT[:, k, t * P:(t + 1) * P],
                        kT[:, k, b0:b0 + n],
                        start=(k == 0), stop=(k == KC - 1))
                nc.scalar.copy(srow[:, b0:b0 + n], pt[:, :n])
            return srow

        def softmax_t(t, srow, ph3, psX):
            L = (t + 1) * P
            nc.vector.tensor_add(
                out=srow[:, t * P:(t + 1) * P],
                in0=srow[:, t * P:(t + 1) * P], in1=cmask[:])
            nmax = ph3.tile([P, 1], F32, tag="nmax")
            nc.vector.tensor_reduce(
                nmax[:], srow[:, :L], axis=mybir.AxisListType.X,
                op=mybir.AluOpType.max, negate=True)
            nbias = ph3.tile([P, 1], F32, tag="nbias")
            nc.vector.tensor_scalar_mul(nbias[:], nmax[:], SCALE)
            prow = ph3.tile([P, T], BF16, tag="prow")
            rsum = ph3.tile([P, 1], F32, tag="rsum")
            nc.scalar.activation(
                prow[:, :L], srow[:, :L], mybir.ActivationFunctionType.Exp,
                bias=nbias[:, :1], scale=SCALE, accum_out=rsum[:, :1])
            rinv = ph3.tile([P, 1], F32, tag="rinv")
            nc.vector.reciprocal(rinv[:], rsum[:])
            nc.vector.tensor_scalar_mul(prow[:, :L], prow[:, :L], rinv[:, :1])
            for s in range(t + 1):
                tp = psX.tile([P, P], BF16, tag="tp")
                nc.tensor.transpose(
                    tp[:], prow[:, s * P:(s + 1) * P], ident_bf[:])
                nc.vector.tensor_copy(pT[s][:, t * P:(t + 1) * P], tp[:])

        def av_block(blk, psX):
            for h in range(KC):
                smax = 4 if blk == 0 else TT
                pt = psX.tile([P, 512], F32, tag="av")
                for s in range(smax):
                    nc.tensor.matmul(
                        pt[:],
                        v_s[:, s, h * P:(h + 1) * P],
                        pT[s][:, blk * 512:(blk + 1) * 512],
                        start=(s == 0), stop=(s == smax - 1))
                nc.scalar.activation(
                    oT8[h // 2][:, h % 2, blk * 512:(blk + 1) * 512],
                    pt[:], mybir.ActivationFunctionType.Identity,
                    scale=O_SCALE)
                if blk == 0:
                    # bf16 copy of the first t-tile for the t0 block
                    nc.vector.tensor_copy(oT0[:, h, :], pt[:, :P])

        def logits_mm(wl8, j, half, lo, vt, psX, lp_bufs):
            c0 = P if half == 0 else 512   # t-tile 0 handled in bf16
            c1 = (half + 1) * 512
            pt = psX.tile([P, 512], F32, tag="lp", bufs=lp_bufs)
            for k in range(KC // 2):
                nc.tensor.matmul(
                    pt[:, :c1 - c0],
                    wl8[:, 2 * k:2 * k + 2, j * P:(j + 1) * P],
                    oT8[k][:, :, c0:c1],
                    start=(k == 0), stop=(k == KC // 2 - 1),
                    perf_mode=mybir.MatmulPerfMode.DoubleRow)
            if with_bias:
                nc.scalar.activation(
                    lo[:, c0:c1], pt[:, :c1 - c0],
                    mybir.ActivationFunctionType.Identity,
                    bias=blm_s[:, vt:vt + 1], scale=DESC)
            else:
                nc.vector.tensor_scalar_mul(
                    lo[:, c0:c1], pt[:, :c1 - c0], DESC)

        def t0_block(wlb, off, w, psX, t0_bufs):
            # logits0[t0, v] = att_out[t0, :] @ W_lm, bf16
            for n0 in range(0, w, 512):
                n = min(512, w - n0)
                p0 = psX.tile([P, 512], F32, tag="t0", bufs=t0_bufs)
                for k in range(KC):
                    nc.tensor.matmul(
                        p0[:, :n],
                        oT0[:, k, :],
                        wlb[:, k, n0:n0 + n],
                        start=(k == 0), stop=(k == KC - 1))
                l0 = out0p.tile([P, 512], BF16, tag="l0")
                # b_lm for these 128 rows is added host-side at assembly
                nc.vector.tensor_copy(l0[:, :n], p0[:, :n])
                nc.sync.dma_start(
                    out0_d.ap()[:, off + n0:off + n0 + n], l0[:, :n])

        los = [out5a.tile([P, T], BF16, tag=f"lo{j}", name=f"lo{j}")
               for j in range(CHUNK // P)]

        with tc.tile_pool(name="ph3", bufs=4) as ph3, \
             tc.tile_pool(name="psX", bufs=2, space="PSUM") as psX:
            wl80 = convert_wlm(wlbs[0], CHUNK)
            # phase 3a: t-tiles 0..3, with staggered W_lm prefetch
            srows = {}
            for t in range(4):
                srows[t] = scores(t, ph3, psX)
                wlbs.append(issue_wlm(offs[len(wlbs)]))
                softmax_t(t, srows[t], ph3, psX)
            # phase 4a: AV for the first t-half -> oT8 blk0 + oT0
            av_block(0, psX)
            # phase 3b with chunk-0 LM work woven between score groups
            fill = {4: [('t0',)],
                    5: [('j', 0), ('j', 1), ('j', 2)],
                    6: [('j', 3), ('j', 4), ('j', 5)],
                    7: [('j', 6), ('j', 7)]}
            for t in range(4, TT):
                srow = scores(t, ph3, psX)
                for item in fill[t]:
                    if item[0] == 't0':
                        t0_block(wlbs[0], 0, CHUNK, psX, 1)
                    else:
                        j = item[1]
                        logits_mm(wl80, j, 0, los[j], j, psX, 1)
                softmax_t(t, srow, ph3, psX)
            # phase 4b: AV for the second t-half
            av_block(1, psX)

        with tc.tile_pool(name="ps5", bufs=6, space="PSUM") as ps5, \
             tc.tile_pool(name="ps5t", bufs=2, space="PSUM") as ps5t:
            # finish chunk 0: half-1 sweep
            for j in range(CHUNK // P):
                logits_mm(wl80, j, 1, los[j], j, ps5, None)
                nc.sync.dma_start(
                    out_d.ap()[j * P:(j + 1) * P, P:], los[j][:, P:])
            for i in range(1, len(offs)):
                off = offs[i]
                w = min(CHUNK, VH - off)
                if i + PREF - 1 < len(offs):
                    wlbs.append(issue_wlm(offs[i + PREF - 1]))
                wlb = wlbs[i]
                wl8 = convert_wlm(wlb, w)
                t0_block(wlb, off, w, ps5t, None)
                for j in range(w // P):
                    vt = (off + j * P) // P
                    lo = out5.tile([P, T], BF16, tag="lo")
                    for half in range(2):
                        logits_mm(wl8, j, half, lo, vt, ps5, None)
                    nc.sync.dma_start(
                        out_d.ap()[vt * P:(vt + 1) * P, P:], lo[:, P:])
        pT_ctx.close()
        ph5_ctx.close()
        att_ctx.close()


def _get_nc(with_bias):
    key = ("nc", with_bias)
    if key not in _CACHE:
        _CACHE[key] = _build(with_bias)
    return _CACHE[key]


def _make_in_maps(idx, W_embed, W_pos, Wq, Wk, Wv, W_lm, b_lm):
    import ml_dtypes

    W_embed = np.ascontiguousarray(W_embed, dtype=np.float32)
    W_pos = np.ascontiguousarray(W_pos, dtype=np.float32)
    halves_w = []
    halves_b = []
    for h in range(2):
        lo = h * VH
        hi = min(VOCAB, lo + VH)
        wl = np.zeros((C, VH), dtype=ml_dtypes.bfloat16)
        wl[:, :hi - lo] = W_lm[:, lo:hi].astype(ml_dtypes.bfloat16)
        bl = np.zeros((VH,), dtype=np.float32)
        bl[:hi - lo] = b_lm[lo:hi]
        halves_w.append(wl)
        halves_b.append(bl)
    in_maps = []
    for core in range(8):
        b = core >> 1
        h = core & 1
        in_maps.append({
            "idx": np.ascontiguousarray(idx[b], dtype=np.int32),
            "W_embed": W_embed,
            "W_pos": W_pos,
            "Wq": np.asarray(Wq).astype(ml_dtypes.bfloat16),
            "Wk": np.asarray(Wk).astype(ml_dtypes.bfloat16),
            "Wv": np.asarray(Wv).astype(ml_dtypes.bfloat16),
            "W_lm": halves_w[h],
            "b_lm": halves_b[h],
        })
    return in_maps


def _run(inputs, trace=False):
    nc = _get_nc(bool(np.any(np.asarray(inputs["b_lm"]))))
    in_maps = _make_in_maps(**inputs)
    res = bass_utils.run_bass_kernel_spmd(
        nc, in_maps, core_ids=list(range(8)), trace=trace)
    B = inputs["idx"].shape[0]
    b_lm = np.asarray(inputs["b_lm"], dtype=np.float32)
    out = np.empty((B, T, VOCAB), dtype=np.float32)
    for core in range(8):
        b = core >> 1
        h = core & 1
        lo = h * VH
        hi = min(VOCAB, lo + VH)
        out[b, :, lo:hi] = res.results[core]["logitsT"][:hi - lo, :].astype(np.float32).T
        # rows 0:P come from the bf16 t0 block (bias added here)
        out[b, :P, lo:hi] = (res.results[core]["logits0"][:, :hi - lo]
                             .astype(np.float32) + b_lm[lo:hi])
    return out, res


def kernel(**inputs):
    out, _ = _run(inputs, trace=False)
    return out



# revision 48
# speedup vs baseline: 1.0121x; 1.0121x over previous
"""Trainium2 Bass kernel for nn_BasicTransformer (B=4, T=1024, C=H=768,
vocab 50257, single-head causal attention + LM head).

Sharding: 8 cores = 4 batches x 2 vocab halves. Each core computes the
full embedding+attention for its batch (duplicated across the vocab pair,
~7% extra FLOPs) and the LM-head matmul for its vocab half. The LM head
(316 of 345 total GFLOP) dominates, so it is computed as
logitsT[v, t] = W_lm[:, v_tile].T @ att_outT with the weight tile
stationary and att_outT streaming from SBUF; W_lm streams from HBM once.

The LM head for tokens 128..1023 runs in fp8 (e4m3) with
perf_mode=DoubleRow (2 contraction rows per PE pass, ~1.44x bf16): W_lm
is converted bf16->fp8 on-chip scaled by 2^9, the attention output is
scaled by 2^10, and the PSUM result is descaled by 2^-19. Tokens 0..127
keep bf16 via a swapped-operand block (att-out stationary, W_lm moving)
because their attention outputs -- averages of few value rows -- are
large, and fp8 error there would breach the rel-err budget. Logits are
stored to HBM in bf16 and upcast host-side. Attention stays in bf16.
"""

import numpy as np

import concourse.bass as bass
import concourse.mybir as mybir
import concourse.tile as tile
from concourse import bacc, bass_utils
from concourse.masks import make_causal_mask, make_identity

P = 128
T = 1024          # sequence length
C = 768           # features == head size
KC = C // P       # 6 contraction chunks
TT = T // P       # 8 token tiles
VOCAB = 50257
VH = 25216        # padded vocab half (= 197 * 128), 2*VH >= VOCAB
VT = VH // P      # 197 vocab tiles per core
SCALE = float(C) ** -0.5
NEG = -1e30

F32 = mybir.dt.float32
F32R = mybir.dt.float32r
BF16 = mybir.dt.bfloat16
F8 = mybir.dt.float8e4
I32 = mybir.dt.int32

W_SCALE = 512.0       # weight pre-scale (keeps e4m3 out of subnormals)
O_SCALE = 1024.0      # on-chip att-out pre-scale
X_SCALE = 256.0       # embedding pre-scale for fp8 q/k projections
DESC = 1.0 / (W_SCALE * O_SCALE)

_CACHE = {}


def _build(with_bias):
    nc = bacc.Bacc("TRN2", target_bir_lowering=False, debug=False)

    idx_d = nc.dram_tensor("idx", [T], I32, kind="ExternalInput")
    wemb_d = nc.dram_tensor("W_embed", [VOCAB, C], F32, kind="ExternalInput")
    wpos_d = nc.dram_tensor("W_pos", [T, C], F32, kind="ExternalInput")
    wq_d = nc.dram_tensor("Wq", [C, C], BF16, kind="ExternalInput")
    wk_d = nc.dram_tensor("Wk", [C, C], BF16, kind="ExternalInput")
    wv_d = nc.dram_tensor("Wv", [C, C], BF16, kind="ExternalInput")
    wlm_d = nc.dram_tensor("W_lm", [C, VH], BF16, kind="ExternalInput")
    blm_d = nc.dram_tensor("b_lm", [VH], F32, kind="ExternalInput")
    out_d = nc.dram_tensor("logitsT", [VH, T], BF16, kind="ExternalOutput")
    out0_d = nc.dram_tensor("logits0", [P, VH], BF16, kind="ExternalOutput")

    with tile.TileContext(nc) as tc:
        _body(tc, nc, idx_d, wemb_d, wpos_d, wq_d, wk_d, wv_d, wlm_d,
              blm_d, out_d, out0_d, with_bias)
    nc.compile()
    return nc


def _body(tc, nc, idx_d, wemb_d, wpos_d, wq_d, wk_d, wv_d, wlm_d, blm_d,
          out_d, out0_d, with_bias):
    from contextlib import ExitStack

    with ExitStack() as ctx:
        const = ctx.enter_context(tc.tile_pool(name="const", bufs=1))

        ident = const.tile([P, P], F32)
        make_identity(nc, ident[:])
        cmask = const.tile([P, P], F32)
        make_causal_mask(nc, cmask[:], mask_val=NEG)
        ident_bf = const.tile([P, P], BF16)
        nc.vector.tensor_copy(ident_bf[:], ident[:])
        blm_s = const.tile([P, VT], F32)

        att_ctx = ExitStack()
        attp = att_ctx.enter_context(tc.tile_pool(name="attp", bufs=1))
        qT = attp.tile([P, KC, T], BF16)      # q transposed  [h, t]
        kT = attp.tile([P, KC, T], BF16)      # k transposed  [h, t]
        v_s = attp.tile([P, TT, C], BF16)     # v             [s, h]
        # dedicated early-address pools for W_lm chunk 0: the streaming ph5
        # pool reuses SBUF released by the phase-1/2 pools, so its first DMA
        # can't start until attention drains; chunk 0 must not wait for that
        wlm0p = att_ctx.enter_context(tc.tile_pool(name="wlm0", bufs=1))
        wl80p = att_ctx.enter_context(tc.tile_pool(name="wl80", bufs=1))
        # att output transposed [h, t], fp8, one tile per h-PAIR so the
        # DoubleRow rhs AP [p, 2, t] comes from a single tile
        oT8 = [const.tile([P, 2, T], F8, name=f"oT8{j}")
               for j in range(KC // 2)]
        # bf16 att output for the first t-tile (t0 block of the LM head)
        oT0 = const.tile([P, KC, P], BF16, name="oT0")

        early_ctx = ExitStack()
        early = early_ctx.enter_context(tc.tile_pool(name="early", bufs=1))
        xT = early.tile([P, KC, T], BF16)     # x transposed  [c, t]
        xT8 = early.tile([P, KC, T], F8)      # scaled fp8 copy for q/k proj

        # ---- phase 1: embedding gather + positional add + transpose ----
        # All critical-path DMAs (idx -> gather, W_pos, Wq/Wk/Wv) are issued
        # ahead of compute so they enqueue before the W_lm prefetch stream.
        with tc.tile_pool(name="ph1", bufs=1) as ph1, \
             tc.tile_pool(name="ps1", bufs=4, space="PSUM") as ps1:
            # Exactly 8 HWDGE DMAs up front — one per semaphore lane, so no
            # issue waits on lane recycling (a big W_pos DMA on a reused lane
            # previously stalled the q/k/v weight loads until ~30us).
            idx_all = ph1.tile([P, TT], I32, tag="idx")
            nc.sync.dma_start(
                idx_all[:], idx_d.ap().rearrange("(t p) -> p t", p=P))
            wq_s = early.tile([P, KC, C], BF16)
            wk_s = early.tile([P, KC, C], BF16)
            wv_s = early.tile([P, KC, C], BF16)
            nc.sync.dma_start(
                wq_s[:], wq_d.ap().rearrange("(k p) c -> p k c", p=P))
            nc.sync.dma_start(
                wk_s[:], wk_d.ap().rearrange("(k p) c -> p k c", p=P))
            xp_all = ph1.tile([P, TT, C], F32, tag="xp")
            xp_r = wpos_d.ap().rearrange("(t p) c -> p t c", p=P)
            nc.scalar.dma_start(xp_all[:, 0:TT // 2], xp_r[:, 0:TT // 2])
            nc.sync.dma_start(
                wv_s[:], wv_d.ap().rearrange("(k p) c -> p k c", p=P))
            nc.scalar.dma_start(xp_all[:, TT // 2:], xp_r[:, TT // 2:])
            nc.sync.dma_start(blm_s[:], blm_d.ap().rearrange("(o p) -> p o", p=P))
            wlb0 = wlm0p.tile([P, KC, 1024], BF16, tag="wlm0")
            nc.scalar.dma_start(
                wlb0[:], wlm_d.ap()[:, 0:1024].rearrange("(k p) n -> p k n", p=P))
            # one gather per t-tile (multi-row offset gathers hang the
            # gpsimd drain; see round-3 trace)
            xgs = []
            for t in range(TT):
                xg = ph1.tile([P, C], F32, tag=f"xg{t}")
                nc.gpsimd.indirect_dma_start(
                    out=xg[:], out_offset=None, in_=wemb_d.ap()[:],
                    in_offset=bass.IndirectOffsetOnAxis(
                        ap=idx_all[:, t:t + 1], axis=0))
                xgs.append(xg)
            # q/k weight fp8 converts early in the scalar stream, ahead of
            # the xT copies, so phase 2 isn't gated on them
            wq8 = early.tile([P, KC, C], F8)
            wk8 = early.tile([P, KC, C], F8)
            nc.scalar.activation(wq8[:], wq_s[:],
                                 mybir.ActivationFunctionType.Identity,
                                 scale=W_SCALE)
            nc.scalar.activation(wk8[:], wk_s[:],
                                 mybir.ActivationFunctionType.Identity,
                                 scale=W_SCALE)
            for t in range(TT):
                xb = ph1.tile([P, C], BF16, tag=f"xb{t}")
                nc.vector.tensor_add(out=xb[:], in0=xgs[t][:], in1=xp_all[:, t])
                for k in range(KC):
                    tp = ps1.tile([P, P], BF16, tag="tp")
                    nc.tensor.transpose(
                        tp[:], xb[:, k * P:(k + 1) * P], ident_bf[:])
                    nc.scalar.copy(xT[:, k, t * P:(t + 1) * P], tp[:])
                    nc.vector.tensor_scalar_mul(
                        xT8[:, k, t * P:(t + 1) * P], tp[:], X_SCALE)

        # ---- phase 2: q/k/v projections (q/k in fp8 DoubleRow) ----
        QK_DESC = 1.0 / (X_SCALE * W_SCALE)
        with tc.tile_pool(name="ps2", bufs=6, space="PSUM") as ps2:
            for half in range(2):
                for (w8, dstT) in ((wq8, qT), (wk8, kT)):
                    for h in range(KC):
                        pt = ps2.tile([P, 512], F32, tag="qk")
                        for k in range(KC // 2):
                            nc.tensor.matmul(
                                pt[:],
                                w8[:, 2 * k:2 * k + 2, h * P:(h + 1) * P],
                                xT8[:, 2 * k:2 * k + 2,
                                    half * 512:(half + 1) * 512],
                                start=(k == 0), stop=(k == KC // 2 - 1),
                                perf_mode=mybir.MatmulPerfMode.DoubleRow)
                        nc.vector.tensor_scalar_mul(
                            dstT[:, h, half * 512:(half + 1) * 512], pt[:],
                            QK_DESC)
            for s in range(TT):
                for (n0, n1) in ((0, 512), (512, 768)):
                    pt = ps2.tile([P, 512], F32, tag="qk")
                    for k in range(KC):
                        nc.tensor.matmul(
                            pt[:, :n1 - n0],
                            xT[:, k, s * P:(s + 1) * P],
                            wv_s[:, k, n0:n1],
                            start=(k == 0), stop=(k == KC - 1))
                    nc.vector.tensor_copy(v_s[:, s, n0:n1], pt[:, :n1 - n0])
        early_ctx.close()

        # streaming pools for W_lm chunks 1+ (chunk 0 lives in the early
        # wlm0/wl80 pools; these reuse SBUF released by the early pools and
        # only become writable once phase 2 drains)
        ph5_ctx = ExitStack()
        ph5 = ph5_ctx.enter_context(tc.tile_pool(name="ph5", bufs=4))
        out5 = ph5_ctx.enter_context(tc.tile_pool(name="out5", bufs=8))
        out5a = ph5_ctx.enter_context(tc.tile_pool(name="out5a", bufs=1))

        pT_ctx = ExitStack()
        pTp = pT_ctx.enter_context(tc.tile_pool(name="pTp", bufs=1))
        # attention probs transposed [s, t], one tile per s-chunk for finer deps
        pT = [pTp.tile([P, T], BF16, tag=f"pT{s}", name=f"pT{s}") for s in range(TT)]

        # zero the strictly-upper (future) blocks of pT that phase-4 matmuls
        # will read but phase 3 never writes
        for s in range(1, TT):
            lo = 0 if s < 4 else 512
            if s * P > lo:
                nc.vector.memset(pT[s][:, lo:s * P].bitcast(mybir.dt.uint16), 0)

        # ---- phases 3-5, interleaved ----
        # t-tile 0 (128 tokens) of the LM head is computed in bf16 with
        # swapped operands (att-out stationary, W_lm moving) because fp8
        # quantization error there is too large relative to the global logit
        # scale; t>=128 runs in fp8 e4m3 with perf_mode=DoubleRow (2
        # contraction rows per PE pass). Phase 4 is split around the second
        # half of phase 3, and chunk-0 LM work is woven between the phase-3
        # score groups so the PE has work during the softmax latency.
        CHUNK = 1024  # vocab columns per W_lm DMA chunk; VH = 24*1024 + 640
        offs = list(range(0, VH, CHUNK))
        PREF = 4      # W_lm chunks in flight beyond chunk 0 (== ph5 bufs)

        wl8p = pT_ctx.enter_context(tc.tile_pool(name="wl8p", bufs=3))
        out0p = pT_ctx.enter_context(tc.tile_pool(name="out0p", bufs=4))

        def issue_wlm(off):
            # on sync: these issue instructions wait for the early-pool SBUF
            # zone to drain, and nothing latency-critical queues behind them
            # on the sync ring (the scalar ring carries softmax EXPs)
            w = min(CHUNK, VH - off)
            wlb = ph5.tile([P, KC, CHUNK], BF16, tag="wlm", name=f"wlb{off}")
            nc.sync.dma_start(
                wlb[:, :, :w],
                wlm_d.ap()[:, off:off + w].rearrange("(k p) n -> p k n", p=P))
            return wlb

        def convert_wlm(wlb, w, pool):
            # pre-scaled fp8 copy for the DoubleRow sweep; split across
            # scalar+vector so the first MM group isn't gated on one long op
            wl8 = pool.tile([P, KC, CHUNK], F8, tag="wl8")
            nc.scalar.activation(
                wl8[:, 0:2, :w], wlb[:, 0:2, :w],
                mybir.ActivationFunctionType.Identity, scale=W_SCALE)
            nc.scalar.activation(
                wl8[:, 2:4, :w], wlb[:, 2:4, :w],
                mybir.ActivationFunctionType.Identity, scale=W_SCALE)
            nc.vector.tensor_scalar_mul(
                wl8[:, 4:6, :w], wlb[:, 4:6, :w], W_SCALE)
            return wl8

        wlbs = [wlb0] + [issue_wlm(offs[i]) for i in range(1, 1 + PREF)]
        wl80 = convert_wlm(wlb0, CHUNK, wl80p)

        def scores(t, ph3, psX):
            L = (t + 1) * P
            srow = ph3.tile([P, T], F32, tag="srow", name=f"srow{t}")
            for b0 in range(0, L, 512):
                n = min(512, L - b0)
                pt = psX.tile([P, 512], F32, tag="sc")
                for k in range(KC):
                    nc.tensor.matmul(
                        pt[:, :n],
                        qT[:, k, t * P:(t + 1) * P],
                        kT[:, k, b0:b0 + n],
                        start=(k == 0), stop=(k == KC - 1))
                nc.scalar.copy(srow[:, b0:b0 + n], pt[:, :n])
            return srow

        def softmax_t(t, srow, ph3, psX):
            L = (t + 1) * P
            nc.vector.tensor_add(
                out=srow[:, t * P:(t + 1) * P],
                in0=srow[:, t * P:(t + 1) * P], in1=cmask[:])
            nmax = ph3.tile([P, 1], F32, tag="nmax")
            nc.vector.tensor_reduce(
                nmax[:], srow[:, :L], axis=mybir.AxisListType.X,
                op=mybir.AluOpType.max, negate=True)
            nbias = ph3.tile([P, 1], F32, tag="nbias")
            nc.vector.tensor_scalar_mul(nbias[:], nmax[:], SCALE)
            prow = ph3.tile([P, T], BF16, tag="prow")
            rsum = ph3.tile([P, 1], F32, tag="rsum")
            nc.scalar.activation(
                prow[:, :L], srow[:, :L], mybir.ActivationFunctionType.Exp,
                bias=nbias[:, :1], scale=SCALE, accum_out=rsum[:, :1])
            rinv = ph3.tile([P, 1], F32, tag="rinv")
            nc.vector.reciprocal(rinv[:], rsum[:])
            nc.vector.tensor_scalar_mul(prow[:, :L], prow[:, :L], rinv[:, :1])
            for s in range(t + 1):
                tp = psX.tile([P, P], BF16, tag="tp")
                nc.tensor.transpose(
                    tp[:], prow[:, s * P:(s + 1) * P], ident_bf[:])
                nc.vector.tensor_copy(pT[s][:, t * P:(t + 1) * P], tp[:])

        def av_block(blk, psX):
            for h in range(KC):
                smax = 4 if blk == 0 else TT
                pt = psX.tile([P, 512], F32, tag="av")
                for s in range(smax):
                    nc.tensor.matmul(
                        pt[:],
                        v_s[:, s, h * P:(h + 1) * P],
                        pT[s][:, blk * 512:(blk + 1) * 512],
                        start=(s == 0), stop=(s == smax - 1))
                nc.scalar.activation(
                    oT8[h // 2][:, h % 2, blk * 512:(blk + 1) * 512],
                    pt[:], mybir.ActivationFunctionType.Identity,
                    scale=O_SCALE)
                if blk == 0:
                    # bf16 copy of the first t-tile for the t0 block
                    nc.vector.tensor_copy(oT0[:, h, :], pt[:, :P])

        def logits_mm(wl8, j, half, lo, vt, psX, lp_bufs):
            c0 = P if half == 0 else 512   # t-tile 0 handled in bf16
            c1 = (half + 1) * 512
            pt = psX.tile([P, 512], F32, tag="lp", bufs=lp_bufs)
            for k in range(KC // 2):
                nc.tensor.matmul(
                    pt[:, :c1 - c0],
                    wl8[:, 2 * k:2 * k + 2, j * P:(j + 1) * P],
                    oT8[k][:, :, c0:c1],
                    start=(k == 0), stop=(k == KC // 2 - 1),
                    perf_mode=mybir.MatmulPerfMode.DoubleRow)
            if with_bias:
                nc.scalar.activation(
                    lo[:, c0:c1], pt[:, :c1 - c0],
                    mybir.ActivationFunctionType.Identity,
                    bias=blm_s[:, vt:vt + 1], scale=DESC)
            else:
                nc.vector.tensor_scalar_mul(
                    lo[:, c0:c1], pt[:, :c1 - c0], DESC)

        def t0_block(wlb, off, w, psX, t0_bufs):
            # logits0[t0, v] = att_out[t0, :] @ W_lm, bf16
            for n0 in range(0, w, 512):
                n = min(512, w - n0)
                p0 = psX.tile([P, 512], F32, tag="t0", bufs=t0_bufs)
                for k in range(KC):
                    nc.tensor.matmul(
                        p0[:, :n],
                        oT0[:, k, :],
                        wlb[:, k, n0:n0 + n],
                        start=(k == 0), stop=(k == KC - 1))
                l0 = out0p.tile([P, 512], BF16, tag="l0")
                # b_lm for these 128 rows is added host-side at assembly
                nc.vector.tensor_copy(l0[:, :n], p0[:, :n])
                nc.sync.dma_start(
                    out0_d.ap()[:, off + n0:off + n0 + n], l0[:, :n])

        los = [out5a.tile([P, T], BF16, tag=f"lo{j}", name=f"lo{j}")
               for j in range(CHUNK // P)]

        with tc.tile_pool(name="ph3", bufs=4) as ph3, \
             tc.tile_pool(name="psX", bufs=2, space="PSUM") as psX:
            # phase 3a: t-tiles 0..3
            srows = {}
            for t in range(4):
                srows[t] = scores(t, ph3, psX)
                softmax_t(t, srows[t], ph3, psX)
            # phase 4a: AV for the first t-half -> oT8 blk0 + oT0
            av_block(0, psX)
            # phase 3b with chunk-0 LM work woven between score groups
            fill = {4: [('t0',)],
                    5: [('j', 0), ('j', 1), ('j', 2)],
                    6: [('j', 3), ('j', 4), ('j', 5)],
                    7: [('j', 6), ('j', 7)]}
            for t in range(4, TT):
                srow = scores(t, ph3, psX)
                for item in fill[t]:
                    if item[0] == 't0':
                        t0_block(wlbs[0], 0, CHUNK, psX, 1)
                    else:
                        j = item[1]
                        logits_mm(wl80, j, 0, los[j], j, psX, 1)
                softmax_t(t, srow, ph3, psX)
            # phase 4b: AV for the second t-half
            av_block(1, psX)

        with tc.tile_pool(name="ps5", bufs=6, space="PSUM") as ps5, \
             tc.tile_pool(name="ps5t", bufs=2, space="PSUM") as ps5t:
            # finish chunk 0: half-1 sweep
            for j in range(CHUNK // P):
                logits_mm(wl80, j, 1, los[j], j, ps5, None)
                nc.sync.dma_start(
                    out_d.ap()[j * P:(j + 1) * P, P:], los[j][:, P:])
            for i in range(1, len(offs)):
                off = offs[i]
                w = min(CHUNK, VH - off)
                if i + PREF < len(offs):
                    wlbs.append(issue_wlm(offs[i + PREF]))
                wlb = wlbs[i]
                wl8 = convert_wlm(wlb, w, wl8p)
                t0_block(wlb, off, w, ps5t, None)
                for j in range(w // P):
                    vt = (off + j * P) // P
                    lo = out5.tile([P, T], BF16, tag="lo")
                    for half in range(2):
                        logits_mm(wl8, j, half, lo, vt, ps5, None)
                    nc.sync.dma_start(
                        out_d.ap()[vt * P:(vt + 1) * P, P:], lo[:, P:])
        pT_ctx.close()
        ph5_ctx.close()
        att_ctx.close()


def _get_nc(with_bias):
    key = ("nc", with_bias)
    if key not in _CACHE:
        _CACHE[key] = _build(with_bias)
    return _CACHE[key]


def _make_in_maps(idx, W_embed, W_pos, Wq, Wk, Wv, W_lm, b_lm):
    import ml_dtypes

    W_embed = np.ascontiguousarray(W_embed, dtype=np.float32)
    W_pos = np.ascontiguousarray(W_pos, dtype=np.float32)
    halves_w = []
    halves_b = []
    for h in range(2):
        lo = h * VH
        hi = min(VOCAB, lo + VH)
        wl = np.zeros((C, VH), dtype=ml_dtypes.bfloat16)
        wl[:, :hi - lo] = W_lm[:, lo:hi].astype(ml_dtypes.bfloat16)
        bl = np.zeros((VH,), dtype=np.float32)
        bl[:hi - lo] = b_lm[lo:hi]
        halves_w.append(wl)
        halves_b.append(bl)
    in_maps = []
    for core in range(8):
        b = core >> 1
        h = core & 1
        in_maps.append({
            "idx": np.ascontiguousarray(idx[b], dtype=np.int32),
            "W_embed": W_embed,
            "W_pos": W_pos,
            "Wq": np.asarray(Wq).astype(ml_dtypes.bfloat16),
            "Wk": np.asarray(Wk).astype(ml_dtypes.bfloat16),
            "Wv": np.asarray(Wv).astype(ml_dtypes.bfloat16),
            "W_lm": halves_w[h],
            "b_lm": halves_b[h],
        })
    return in_maps


def _run(inputs, trace=False):
    nc = _get_nc(bool(np.any(np.asarray(inputs["b_lm"]))))
    in_maps = _make_in_maps(**inputs)
    res = bass_utils.run_bass_kernel_spmd(
        nc, in_maps, core_ids=list(range(8)), trace=trace)
    B = inputs["idx"].shape[0]
    b_lm = np.asarray(inputs["b_lm"], dtype=np.float32)
    out = np.empty((B, T, VOCAB), dtype=np.float32)
    for core in range(8):
        b = core >> 1
        h = core & 1
        lo = h * VH
        hi = min(VOCAB, lo + VH)
        out[b, :, lo:hi] = res.results[core]["logitsT"][:hi - lo, :].astype(np.float32).T
        # rows 0:P come from the bf16 t0 block (bias added here)
        out[b, :P, lo:hi] = (res.results[core]["logits0"][:, :hi - lo]
                             .astype(np.float32) + b_lm[lo:hi])
    return out, res


def kernel(**inputs):
    out, _ = _run(inputs, trace=False)
    return out



# revision 53
# speedup vs baseline: 1.0261x; 1.0138x over previous
"""Trainium2 Bass kernel for nn_BasicTransformer (B=4, T=1024, C=H=768,
vocab 50257, single-head causal attention + LM head).

Sharding: 8 cores = 4 batches x 2 vocab halves. Each core computes the
full embedding+attention for its batch (duplicated across the vocab pair,
~7% extra FLOPs) and the LM-head matmul for its vocab half. The LM head
(316 of 345 total GFLOP) dominates, so it is computed as
logitsT[v, t] = W_lm[:, v_tile].T @ att_outT with the weight tile
stationary and att_outT streaming from SBUF; W_lm streams from HBM once.

The LM head for tokens 128..1023 runs in fp8 (e4m3) with
perf_mode=DoubleRow (2 contraction rows per PE pass, ~1.44x bf16): W_lm
is converted bf16->fp8 on-chip scaled by 2^9, the attention output is
scaled by 2^10, and the PSUM result is descaled by 2^-19. Tokens 0..127
keep bf16 via a swapped-operand block (att-out stationary, W_lm moving)
because their attention outputs -- averages of few value rows -- are
large, and fp8 error there would breach the rel-err budget. Logits are
stored to HBM in bf16 and upcast host-side. Attention stays in bf16.
"""

import numpy as np

import concourse.bass as bass
import concourse.mybir as mybir
import concourse.tile as tile
from concourse import bacc, bass_utils
from concourse.masks import make_causal_mask, make_identity

P = 128
T = 1024          # sequence length
C = 768           # features == head size
KC = C // P       # 6 contraction chunks
TT = T // P       # 8 token tiles
VOCAB = 50257
VH = 25216        # padded vocab half (= 197 * 128), 2*VH >= VOCAB
VT = VH // P      # 197 vocab tiles per core
SCALE = float(C) ** -0.5
NEG = -1e30

F32 = mybir.dt.float32
F32R = mybir.dt.float32r
BF16 = mybir.dt.bfloat16
F8 = mybir.dt.float8e4
I32 = mybir.dt.int32

W_SCALE = 512.0       # weight pre-scale (keeps e4m3 out of subnormals)
O_SCALE = 1024.0      # on-chip att-out pre-scale
X_SCALE = 256.0       # embedding pre-scale for fp8 q/k projections
DESC = 1.0 / (W_SCALE * O_SCALE)

_CACHE = {}


def _build(with_bias):
    nc = bacc.Bacc("TRN2", target_bir_lowering=False, debug=False)

    idx_d = nc.dram_tensor("idx", [T], I32, kind="ExternalInput")
    wemb_d = nc.dram_tensor("W_embed", [VOCAB, C], F32, kind="ExternalInput")
    wpos_d = nc.dram_tensor("W_pos", [T, C], F32, kind="ExternalInput")
    wq_d = nc.dram_tensor("Wq", [C, C], BF16, kind="ExternalInput")
    wk_d = nc.dram_tensor("Wk", [C, C], BF16, kind="ExternalInput")
    wv_d = nc.dram_tensor("Wv", [C, C], BF16, kind="ExternalInput")
    wlm_d = nc.dram_tensor("W_lm", [C, VH], BF16, kind="ExternalInput")
    blm_d = nc.dram_tensor("b_lm", [VH], F32, kind="ExternalInput")
    out_d = nc.dram_tensor("logitsT", [VH, T], BF16, kind="ExternalOutput")
    out0_d = nc.dram_tensor("logits0", [P, VH], BF16, kind="ExternalOutput")

    with tile.TileContext(nc) as tc:
        _body(tc, nc, idx_d, wemb_d, wpos_d, wq_d, wk_d, wv_d, wlm_d,
              blm_d, out_d, out0_d, with_bias)
    nc.compile()
    return nc


def _body(tc, nc, idx_d, wemb_d, wpos_d, wq_d, wk_d, wv_d, wlm_d, blm_d,
          out_d, out0_d, with_bias):
    from contextlib import ExitStack

    with ExitStack() as ctx:
        const = ctx.enter_context(tc.tile_pool(name="const", bufs=1))

        ident = const.tile([P, P], F32)
        make_identity(nc, ident[:])
        cmask = const.tile([P, P], F32)
        make_causal_mask(nc, cmask[:], mask_val=NEG)
        ident_bf = const.tile([P, P], BF16)
        nc.vector.tensor_copy(ident_bf[:], ident[:])
        blm_s = const.tile([P, VT], F32)

        att_ctx = ExitStack()
        attp = att_ctx.enter_context(tc.tile_pool(name="attp", bufs=1))
        qT = attp.tile([P, KC, T], BF16)      # q transposed  [h, t]
        kT = attp.tile([P, KC, T], BF16)      # k transposed  [h, t]
        v_s = attp.tile([P, TT, C], BF16)     # v             [s, h]
        # dedicated early-address pools for W_lm chunk 0: the streaming ph5
        # pool reuses SBUF released by the phase-1/2 pools, so its first DMA
        # can't start until attention drains; chunk 0 must not wait for that
        wlm0p = att_ctx.enter_context(tc.tile_pool(name="wlm0", bufs=1))
        wl80p = att_ctx.enter_context(tc.tile_pool(name="wl80", bufs=1))
        # fp8 W_lm chunk copies also at early addresses: in recycled SBUF the
        # chunk-1 convert would wait for the attention pools to drain
        wl8p = att_ctx.enter_context(tc.tile_pool(name="wl8p", bufs=3))
        # att output transposed [h, t], fp8, one tile per h-PAIR so the
        # DoubleRow rhs AP [p, 2, t] comes from a single tile
        oT8 = [const.tile([P, 2, T], F8, name=f"oT8{j}")
               for j in range(KC // 2)]
        # bf16 att output for the first t-tile (t0 block of the LM head)
        oT0 = const.tile([P, KC, P], BF16, name="oT0")

        early_ctx = ExitStack()
        early = early_ctx.enter_context(tc.tile_pool(name="early", bufs=1))
        xT = early.tile([P, KC, T], BF16)     # x transposed  [c, t]
        xT8 = early.tile([P, KC, T], F8)      # scaled fp8 copy for q/k proj

        # ---- phase 1: embedding gather + positional add + transpose ----
        # All critical-path DMAs (idx -> gather, W_pos, Wq/Wk/Wv) are issued
        # ahead of compute so they enqueue before the W_lm prefetch stream.
        with tc.tile_pool(name="ph1", bufs=1) as ph1, \
             tc.tile_pool(name="ps1", bufs=4, space="PSUM") as ps1:
            # Exactly 8 HWDGE DMAs up front — one per semaphore lane, so no
            # issue waits on lane recycling (a big W_pos DMA on a reused lane
            # previously stalled the q/k/v weight loads until ~30us).
            idx_all = ph1.tile([P, TT], I32, tag="idx")
            nc.sync.dma_start(
                idx_all[:], idx_d.ap().rearrange("(t p) -> p t", p=P))
            wq_s = early.tile([P, KC, C], BF16)
            wk_s = early.tile([P, KC, C], BF16)
            wv_s = early.tile([P, KC, C], BF16)
            nc.sync.dma_start(
                wq_s[:], wq_d.ap().rearrange("(k p) c -> p k c", p=P))
            nc.sync.dma_start(
                wk_s[:], wk_d.ap().rearrange("(k p) c -> p k c", p=P))
            xp_all = ph1.tile([P, TT, C], F32, tag="xp")
            xp_r = wpos_d.ap().rearrange("(t p) c -> p t c", p=P)
            nc.scalar.dma_start(xp_all[:, 0:TT // 2], xp_r[:, 0:TT // 2])
            nc.sync.dma_start(
                wv_s[:], wv_d.ap().rearrange("(k p) c -> p k c", p=P))
            nc.scalar.dma_start(xp_all[:, TT // 2:], xp_r[:, TT // 2:])
            nc.sync.dma_start(blm_s[:], blm_d.ap().rearrange("(o p) -> p o", p=P))
            wlb0 = wlm0p.tile([P, KC, 1024], BF16, tag="wlm0")
            nc.scalar.dma_start(
                wlb0[:], wlm_d.ap()[:, 0:1024].rearrange("(k p) n -> p k n", p=P))
            # one gather per t-tile (multi-row offset gathers hang the
            # gpsimd drain; see round-3 trace)
            xgs = []
            for t in range(TT):
                xg = ph1.tile([P, C], F32, tag=f"xg{t}")
                nc.gpsimd.indirect_dma_start(
                    out=xg[:], out_offset=None, in_=wemb_d.ap()[:],
                    in_offset=bass.IndirectOffsetOnAxis(
                        ap=idx_all[:, t:t + 1], axis=0))
                xgs.append(xg)
            for t in range(TT):
                xb = ph1.tile([P, C], BF16, tag=f"xb{t}")
                nc.vector.tensor_add(out=xb[:], in0=xgs[t][:], in1=xp_all[:, t])
                for k in range(KC):
                    tp = ps1.tile([P, P], BF16, tag="tp")
                    nc.tensor.transpose(
                        tp[:], xb[:, k * P:(k + 1) * P], ident_bf[:])
                    nc.vector.tensor_copy(xT[:, k, t * P:(t + 1) * P], tp[:])
                    nc.vector.tensor_scalar_mul(
                        xT8[:, k, t * P:(t + 1) * P], tp[:], X_SCALE)

        # ---- phase 2: q/k/v projections (q/k in fp8 DoubleRow) ----
        wq8 = early.tile([P, KC, C], F8)
        wk8 = early.tile([P, KC, C], F8)
        nc.scalar.activation(wq8[:], wq_s[:],
                             mybir.ActivationFunctionType.Identity,
                             scale=W_SCALE)
        nc.scalar.activation(wk8[:], wk_s[:],
                             mybir.ActivationFunctionType.Identity,
                             scale=W_SCALE)
        QK_DESC = 1.0 / (X_SCALE * W_SCALE)
        with tc.tile_pool(name="ps2", bufs=6, space="PSUM") as ps2:
            for half in range(2):
                for (w8, dstT) in ((wq8, qT), (wk8, kT)):
                    for h in range(KC):
                        pt = ps2.tile([P, 512], F32, tag="qk")
                        for k in range(KC // 2):
                            nc.tensor.matmul(
                                pt[:],
                                w8[:, 2 * k:2 * k + 2, h * P:(h + 1) * P],
                                xT8[:, 2 * k:2 * k + 2,
                                    half * 512:(half + 1) * 512],
                                start=(k == 0), stop=(k == KC // 2 - 1),
                                perf_mode=mybir.MatmulPerfMode.DoubleRow)
                        nc.vector.tensor_scalar_mul(
                            dstT[:, h, half * 512:(half + 1) * 512], pt[:],
                            QK_DESC)
            for s in range(TT):
                for (n0, n1) in ((0, 512), (512, 768)):
                    pt = ps2.tile([P, 512], F32, tag="qk")
                    for k in range(KC):
                        nc.tensor.matmul(
                            pt[:, :n1 - n0],
                            xT[:, k, s * P:(s + 1) * P],
                            wv_s[:, k, n0:n1],
                            start=(k == 0), stop=(k == KC - 1))
                    nc.vector.tensor_copy(v_s[:, s, n0:n1], pt[:, :n1 - n0])
        early_ctx.close()

        # streaming pools for W_lm chunks 1+ (chunk 0 lives in the early
        # wlm0/wl80 pools; these reuse SBUF released by the early pools and
        # only become writable once phase 2 drains)
        ph5_ctx = ExitStack()
        ph5 = ph5_ctx.enter_context(tc.tile_pool(name="ph5", bufs=4))
        out5 = ph5_ctx.enter_context(tc.tile_pool(name="out5", bufs=8))
        out5a = ph5_ctx.enter_context(tc.tile_pool(name="out5a", bufs=1))

        pT_ctx = ExitStack()
        pTp = pT_ctx.enter_context(tc.tile_pool(name="pTp", bufs=1))
        # attention probs transposed [s, t], one tile per s-chunk for finer deps
        pT = [pTp.tile([P, T], BF16, tag=f"pT{s}", name=f"pT{s}") for s in range(TT)]

        # zero the strictly-upper (future) blocks of pT that phase-4 matmuls
        # will read but phase 3 never writes
        for s in range(1, TT):
            lo = 0 if s < 4 else 512
            if s * P > lo:
                nc.vector.memset(pT[s][:, lo:s * P].bitcast(mybir.dt.uint16), 0)

        # ---- phases 3-5, interleaved ----
        # t-tile 0 (128 tokens) of the LM head is computed in bf16 with
        # swapped operands (att-out stationary, W_lm moving) because fp8
        # quantization error there is too large relative to the global logit
        # scale; t>=128 runs in fp8 e4m3 with perf_mode=DoubleRow (2
        # contraction rows per PE pass). Phase 4 is split around the second
        # half of phase 3, and chunk-0 LM work is woven between the phase-3
        # score groups so the PE has work during the softmax latency.
        CHUNK = 1024  # vocab columns per W_lm DMA chunk; VH = 24*1024 + 640
        offs = list(range(0, VH, CHUNK))
        PREF = 4      # W_lm chunks in flight beyond chunk 0 (== ph5 bufs)

        out0p = pT_ctx.enter_context(tc.tile_pool(name="out0p", bufs=4))

        def issue_wlm(off):
            # on sync: these issue instructions wait for the early-pool SBUF
            # zone to drain, and nothing latency-critical queues behind them
            # on the sync ring (the scalar ring carries softmax EXPs)
            w = min(CHUNK, VH - off)
            wlb = ph5.tile([P, KC, CHUNK], BF16, tag="wlm", name=f"wlb{off}")
            nc.sync.dma_start(
                wlb[:, :, :w],
                wlm_d.ap()[:, off:off + w].rearrange("(k p) n -> p k n", p=P))
            return wlb

        def convert_wlm(wlb, w, pool):
            # pre-scaled fp8 copy for the DoubleRow sweep; split across
            # scalar+vector so the first MM group isn't gated on one long op
            wl8 = pool.tile([P, KC, CHUNK], F8, tag="wl8")
            nc.scalar.activation(
                wl8[:, 0:2, :w], wlb[:, 0:2, :w],
                mybir.ActivationFunctionType.Identity, scale=W_SCALE)
            nc.scalar.activation(
                wl8[:, 2:4, :w], wlb[:, 2:4, :w],
                mybir.ActivationFunctionType.Identity, scale=W_SCALE)
            nc.vector.tensor_scalar_mul(
                wl8[:, 4:6, :w], wlb[:, 4:6, :w], W_SCALE)
            return wl8

        wlbs = [wlb0] + [issue_wlm(offs[i]) for i in range(1, 1 + PREF)]
        wl80 = convert_wlm(wlb0, CHUNK, wl80p)

        def scores(t, ph3, psX):
            L = (t + 1) * P
            srow = ph3.tile([P, T], F32, tag="srow", name=f"srow{t}")
            for b0 in range(0, L, 512):
                n = min(512, L - b0)
                pt = psX.tile([P, 512], F32, tag="sc")
                for k in range(KC):
                    nc.tensor.matmul(
                        pt[:, :n],
                        qT[:, k, t * P:(t + 1) * P],
                        kT[:, k, b0:b0 + n],
                        start=(k == 0), stop=(k == KC - 1))
                nc.scalar.copy(srow[:, b0:b0 + n], pt[:, :n])
            return srow

        def softmax_t(t, srow, ph3, psX):
            L = (t + 1) * P
            nc.vector.tensor_add(
                out=srow[:, t * P:(t + 1) * P],
                in0=srow[:, t * P:(t + 1) * P], in1=cmask[:])
            nmax = ph3.tile([P, 1], F32, tag="nmax")
            nc.vector.tensor_reduce(
                nmax[:], srow[:, :L], axis=mybir.AxisListType.X,
                op=mybir.AluOpType.max, negate=True)
            nbias = ph3.tile([P, 1], F32, tag="nbias")
            nc.vector.tensor_scalar_mul(nbias[:], nmax[:], SCALE)
            prow = ph3.tile([P, T], BF16, tag="prow")
            rsum = ph3.tile([P, 1], F32, tag="rsum")
            nc.scalar.activation(
                prow[:, :L], srow[:, :L], mybir.ActivationFunctionType.Exp,
                bias=nbias[:, :1], scale=SCALE, accum_out=rsum[:, :1])
            rinv = ph3.tile([P, 1], F32, tag="rinv")
            nc.vector.reciprocal(rinv[:], rsum[:])
            nc.vector.tensor_scalar_mul(prow[:, :L], prow[:, :L], rinv[:, :1])
            for s in range(t + 1):
                tp = psX.tile([P, P], BF16, tag="tp")
                nc.tensor.transpose(
                    tp[:], prow[:, s * P:(s + 1) * P], ident_bf[:])
                nc.vector.tensor_copy(pT[s][:, t * P:(t + 1) * P], tp[:])

        def av_block(blk, psX):
            for h in range(KC):
                smax = 4 if blk == 0 else TT
                pt = psX.tile([P, 512], F32, tag="av")
                for s in range(smax):
                    nc.tensor.matmul(
                        pt[:],
                        v_s[:, s, h * P:(h + 1) * P],
                        pT[s][:, blk * 512:(blk + 1) * 512],
                        start=(s == 0), stop=(s == smax - 1))
                nc.scalar.activation(
                    oT8[h // 2][:, h % 2, blk * 512:(blk + 1) * 512],
                    pt[:], mybir.ActivationFunctionType.Identity,
                    scale=O_SCALE)
                if blk == 0:
                    # bf16 copy of the first t-tile for the t0 block (scalar:
                    # the vector stream is the busy one at this boundary)
                    nc.scalar.copy(oT0[:, h, :], pt[:, :P])

        def logits_mm(wl8, j, half, lo, vt, psX, lp_bufs):
            c0 = P if half == 0 else 512   # t-tile 0 handled in bf16
            c1 = (half + 1) * 512
            pt = psX.tile([P, 512], F32, tag="lp", bufs=lp_bufs)
            for k in range(KC // 2):
                nc.tensor.matmul(
                    pt[:, :c1 - c0],
                    wl8[:, 2 * k:2 * k + 2, j * P:(j + 1) * P],
                    oT8[k][:, :, c0:c1],
                    start=(k == 0), stop=(k == KC // 2 - 1),
                    perf_mode=mybir.MatmulPerfMode.DoubleRow)
            if with_bias:
                nc.scalar.activation(
                    lo[:, c0:c1], pt[:, :c1 - c0],
                    mybir.ActivationFunctionType.Identity,
                    bias=blm_s[:, vt:vt + 1], scale=DESC)
            else:
                nc.vector.tensor_scalar_mul(
                    lo[:, c0:c1], pt[:, :c1 - c0], DESC)

        def t0_block(wlb, off, w, psX, t0_bufs):
            # logits0[t0, v] = att_out[t0, :] @ W_lm, bf16
            for n0 in range(0, w, 512):
                n = min(512, w - n0)
                p0 = psX.tile([P, 512], F32, tag="t0", bufs=t0_bufs)
                for k in range(KC):
                    nc.tensor.matmul(
                        p0[:, :n],
                        oT0[:, k, :],
                        wlb[:, k, n0:n0 + n],
                        start=(k == 0), stop=(k == KC - 1))
                l0 = out0p.tile([P, 512], BF16, tag="l0")
                # b_lm for these 128 rows is added host-side at assembly
                nc.vector.tensor_copy(l0[:, :n], p0[:, :n])
                nc.sync.dma_start(
                    out0_d.ap()[:, off + n0:off + n0 + n], l0[:, :n])

        los = [out5a.tile([P, T], BF16, tag=f"lo{j}", name=f"lo{j}")
               for j in range(CHUNK // P)]

        with tc.tile_pool(name="ph3", bufs=4) as ph3, \
             tc.tile_pool(name="psX", bufs=2, space="PSUM") as psX:
            # phase 3a: t-tiles 0..3
            srows = {}
            for t in range(4):
                srows[t] = scores(t, ph3, psX)
                softmax_t(t, srows[t], ph3, psX)
            # phase 4a: AV for the first t-half -> oT8 blk0 + oT0
            av_block(0, psX)
            # phase 3b with chunk-0 LM work woven between score groups
            fill = {4: [('t0',)],
                    5: [('j', 0), ('j', 1), ('j', 2)],
                    6: [('j', 3), ('j', 4), ('j', 5)],
                    7: [('j', 6), ('j', 7)]}
            for t in range(4, TT):
                srow = scores(t, ph3, psX)
                for item in fill[t]:
                    if item[0] == 't0':
                        t0_block(wlbs[0], 0, CHUNK, psX, 1)
                    else:
                        j = item[1]
                        logits_mm(wl80, j, 0, los[j], j, psX, 1)
                softmax_t(t, srow, ph3, psX)
            # phase 4b: AV for the second t-half
            av_block(1, psX)

        with tc.tile_pool(name="ps5", bufs=6, space="PSUM") as ps5, \
             tc.tile_pool(name="ps5t", bufs=2, space="PSUM") as ps5t:
            # finish chunk 0: half-1 sweep
            for j in range(CHUNK // P):
                logits_mm(wl80, j, 1, los[j], j, ps5, None)
                nc.sync.dma_start(
                    out_d.ap()[j * P:(j + 1) * P, P:], los[j][:, P:])
            for i in range(1, len(offs)):
                off = offs[i]
                w = min(CHUNK, VH - off)
                if i + PREF < len(offs):
                    wlbs.append(issue_wlm(offs[i + PREF]))
                wlb = wlbs[i]
                wl8 = convert_wlm(wlb, w, wl8p)
                t0_block(wlb, off, w, ps5t, None)
                # last chunks: DMA each t-half as soon as its descale lands,
                # so the final output drain overlaps the closing matmuls
                split_out = i >= len(offs) - 3
                for j in range(w // P):
                    vt = (off + j * P) // P
                    lo = out5.tile([P, T], BF16, tag="lo")
                    for half in range(2):
                        logits_mm(wl8, j, half, lo, vt, ps5, None)
                        if split_out:
                            c0 = P if half == 0 else 512
                            nc.sync.dma_start(
                                out_d.ap()[vt * P:(vt + 1) * P,
                                           c0:(half + 1) * 512],
                                lo[:, c0:(half + 1) * 512])
                    if not split_out:
                        nc.sync.dma_start(
                            out_d.ap()[vt * P:(vt + 1) * P, P:], lo[:, P:])
        pT_ctx.close()
        ph5_ctx.close()
        att_ctx.close()


def _get_nc(with_bias):
    key = ("nc", with_bias)
    if key not in _CACHE:
        _CACHE[key] = _build(with_bias)
    return _CACHE[key]


def _make_in_maps(idx, W_embed, W_pos, Wq, Wk, Wv, W_lm, b_lm):
    import ml_dtypes

    W_embed = np.ascontiguousarray(W_embed, dtype=np.float32)
    W_pos = np.ascontiguousarray(W_pos, dtype=np.float32)
    halves_w = []
    halves_b = []
    for h in range(2):
        lo = h * VH
        hi = min(VOCAB, lo + VH)
        wl = np.zeros((C, VH), dtype=ml_dtypes.bfloat16)
        wl[:, :hi - lo] = W_lm[:, lo:hi].astype(ml_dtypes.bfloat16)
        bl = np.zeros((VH,), dtype=np.float32)
        bl[:hi - lo] = b_lm[lo:hi]
        halves_w.append(wl)
        halves_b.append(bl)
    in_maps = []
    for core in range(8):
        b = core >> 1
        h = core & 1
        in_maps.append({
            "idx": np.ascontiguousarray(idx[b], dtype=np.int32),
            "W_embed": W_embed,
            "W_pos": W_pos,
            "Wq": np.asarray(Wq).astype(ml_dtypes.bfloat16),
            "Wk": np.asarray(Wk).astype(ml_dtypes.bfloat16),
            "Wv": np.asarray(Wv).astype(ml_dtypes.bfloat16),
            "W_lm": halves_w[h],
            "b_lm": halves_b[h],
        })
    return in_maps


def _run(inputs, trace=False):
    nc = _get_nc(bool(np.any(np.asarray(inputs["b_lm"]))))
    in_maps = _make_in_maps(**inputs)
    res = bass_utils.run_bass_kernel_spmd(
        nc, in_maps, core_ids=list(range(8)), trace=trace)
    B = inputs["idx"].shape[0]
    b_lm = np.asarray(inputs["b_lm"], dtype=np.float32)
    out = np.empty((B, T, VOCAB), dtype=np.float32)
    for core in range(8):
        b = core >> 1
        h = core & 1
        lo = h * VH
        hi = min(VOCAB, lo + VH)
        out[b, :, lo:hi] = res.results[core]["logitsT"][:hi - lo, :].astype(np.float32).T
        # rows 0:P come from the bf16 t0 block (bias added here)
        out[b, :P, lo:hi] = (res.results[core]["logits0"][:, :hi - lo]
                             .astype(np.float32) + b_lm[lo:hi])
    return out, res


def kernel(**inputs):
    out, _ = _run(inputs, trace=False)
    return out

